# revision 38
# baseline (speedup 1.0000x reference)
"""AxialChannelAttention TRN2 Bass kernel.

Full inputs: x [16,256,128,128] f32, w1 [64,256], w2 [256,64].
Sharding: data-parallel over batch, 2 samples per core on 8 cores.

Per-core dataflow (read-once/write-once HBM):
  - x loaded as 16 h-quarter tiles [128, 32, 128] via gpsimd SWDGE DMAs
    that cast f32->f16 on the fly: halves SBUF (both samples fully
    resident, no slot-recycling stalls) and keeps loads on a separate DMA
    queue from the output stores (no head-of-line blocking on the SP
    HWDGE queue). Loads are emitted 8 upfront + 8 during b0's pools so
    the default 1024-descriptor SWDGE ring never throttles (a bigger
    ring would eat per-partition SBUF).
  - max pools: pairwise tensor_tensor max trees on DVE (f16 2x_1p mode,
    0.52 ns/elem vs 1.04 for the 1x TensorReduce) with an in-place
    scratch tile per axis. All tree levels stay on DVE: the Pool
    engine's software TensorTensor handles 4-byte dtypes only, and
    TensorReduce/TensorScalarPtr have no DVE fast modes at all.
  - mean pools: PE identity-matmul in f16 with step-0 PSUM out APs
    accumulating 4 h-rows / 16 w-cols per matmul (f32 PSUM accumulate).
    The four accumulators keep separate PSUM banks: concurrent
    accumulation groups sharing one bank corrupt each other on HW.
  - u1s/u2s = w1 @ pools (PE, exact fp32, both branches on 128
    partitions), copied to SBUF on ACT (GPSIMD cannot touch PSUM).
  - gate per 2048-column block, software-pipelined in two stages:
    stage 1 = broadcast-add (GPSIMD TT, step-0 APs) into abi, leaky-relu
    (ACT Prelu) into a separate f32r abo tile; stage 2 = w2cat matmul
    (PE f32r), sigmoid (ACT from PSUM per 512), out = x*(1+s) (DVE
    scalar_tensor_tensor into the sigmoid tile, reading the f16 x),
    DMA out per block (SP HWDGE). Splitting abi/abo and emitting stage 1
    of block k+1 before stage 2 of block k keeps the monotone
    engine-counter semaphores that guard pool-slot reuse from chaining
    each block onto the previous block's sigmoids.
  - emission: b0 pools, then b1's tree pairs alternating with b0's gate
    stage-2 on DVE (one stage-2 block per two trees) so the STTs and the
    stores behind them start ~30us earlier than a pure phase ordering,
    while b1's last tree still lands before ACT finishes b0's gate work
    (u(b1) never stalls ACT). DVE ends up the near-saturated critical
    path (~154us busy of ~204us total). Separate scw/sch scratch tiles
    (not one shared tile) matter: sharing serializes the next tile's
    maxw tree behind the current tile's maxh and costs ~18us.

TimelineSim per-core: ~204.1us (baseline f32r kernel: 293.1us). Engine
busy: DVE ~154us (trees + final mult), ACT ~115us (prelu/sigmoid),
PE ~100us (mean pools + gate matmuls), GPSIMD ~84us (bcast + SWDGE
issue), DMA ~140us modeled / ~187us real-roofline (67MB at ~360GB/s).

f16 x introduces ~5e-4 relative rounding on the pools and the final
multiply; measured end-to-end relative error on HW: 7.1e-4 vs the fp32
reference (threshold 2e-2).
"""
import sys
import numpy as np

if "/opt/trn_rl_repo" not in sys.path:
    sys.path.insert(0, "/opt/trn_rl_repo")

B, C, H, W = 16, 256, 128, 128
CR, P = 64, 128
NCORES = 8
BL = B // NCORES          # samples per core
NEG = 0.01                # leaky relu slope
CT = C // P               # 2 c-tiles
NQ = 4                    # h-quarter tiles per (sample, c-tile)
QS = H // NQ              # 32 h-rows per x tile
NBLK = 8                  # gate blocks per sample (16 h-rows each)
BH = H // NBLK            # 16
NSUB = (BH * W) // 512    # 4 psum sub-blocks per gate block

_nc_cache = None


def _build_nc():
    import concourse.bacc as bacc
    import concourse.bass as bass
    import concourse.tile as tile
    from concourse import mybir
    from concourse.masks import make_identity

    f32 = mybir.dt.float32
    f32r = mybir.dt.float32r
    f16 = mybir.dt.float16
    Alu = mybir.AluOpType
    Act = mybir.ActivationFunctionType
    X = mybir.AxisListType.X

    # default 1024-descriptor SWDGE ring (the carveout eats per-partition
    # SBUF); loads are emitted 8 upfront + 8 interleaved so at most 8 are
    # ever in flight and the ring never throttles
    nc = bacc.Bacc()
    xd = nc.dram_tensor("x", [BL, C, H, W], f32, kind="ExternalInput")
    w1d = nc.dram_tensor("w1", [CR, C], f32, kind="ExternalInput")
    w2d = nc.dram_tensor("w2", [C, CR], f32, kind="ExternalInput")
    od = nc.dram_tensor("out", [BL, C, H, W], f32, kind="ExternalOutput")

    xv = xd[:].rearrange("b (ct cp) h w -> b ct cp h w", ct=CT)
    ov = od[:].rearrange("b (ct cp) h w -> b ct cp h w", ct=CT)

    def bcast_ap(t2d, n_rep, inner_last):
        if inner_last:
            return bass.AP(tensor=t2d.tensor, offset=t2d.offset,
                           ap=[list(t2d.ap[0]), [0, n_rep], list(t2d.ap[1])])
        return bass.AP(tensor=t2d.tensor, offset=t2d.offset,
                       ap=[list(t2d.ap[0]), list(t2d.ap[1]), [0, n_rep]])

    def step0_out(psl, n_rep, inner):
        return bass.AP(tensor=psl.tensor, offset=psl.offset,
                       ap=[list(psl.ap[0]), [0, n_rep], [1, inner]])

    with tile.TileContext(nc) as tc:
        with tc.tile_pool(name="const", bufs=1) as cst, \
             tc.tile_pool(name="xp", bufs=16) as xp, \
             tc.tile_pool(name="scw", bufs=1) as scw, \
             tc.tile_pool(name="sch", bufs=1) as sch, \
             tc.tile_pool(name="pool", bufs=3) as pl, \
             tc.tile_pool(name="gate_i", bufs=2) as gti, \
             tc.tile_pool(name="gate_o", bufs=2) as gto, \
             tc.tile_pool(name="sig", bufs=3) as sg, \
             tc.tile_pool(name="avh_ps", bufs=2, space="PSUM") as avhp, \
             tc.tile_pool(name="avw_ps", bufs=2, space="PSUM") as avwp, \
             tc.tile_pool(name="u_ps", bufs=1, space="PSUM") as upsp, \
             tc.tile_pool(name="g_ps", bufs=3, space="PSUM") as gpsp:

            ident = cst.tile([P, P], f32)
            make_identity(nc, ident)
            ident16 = cst.tile([P, P], f16)
            nc.scalar.copy(ident16, ident)
            # pre-warm the ACT function-table set (Copy/Prelu/Sigmoid):
            # the lazy LoadActFuncSet (~1.3us) otherwise lands in the first
            # gate block's critical chain
            warm = cst.tile([P, 4], f32)
            nc.scalar.activation(out=warm, in_=ident[:, 0:4],
                                 func=Act.Prelu, bias=0.0, scale=1.0,
                                 alpha=NEG)
            nc.scalar.activation(out=warm, in_=ident[:, 0:4],
                                 func=Act.Sigmoid, bias=0.0, scale=1.0)
            # weights: contiguous natural-layout DMAs + on-chip PE transpose
            # (strided 4-byte gather DMAs would cost ~3.6us each at the head
            # of the DMA queue)
            w1T = cst.tile([P, CT, CR], f32)
            w2cat = cst.tile([P, CT, P], f32r)
            w1nat = cst.tile([CR, C], f32)
            nc.sync.dma_start(out=w1nat, in_=w1d[:])
            w2nat = cst.tile([P, CT, CR], f32)
            w2vn = w2d[:].rearrange("(ct cp) r -> ct cp r", ct=CT)
            for ci in range(CT):
                nc.sync.dma_start(out=w2nat[:, ci, :], in_=w2vn[ci])
            for ci in range(CT):
                tp1 = upsp.tile([P, CR], f32, tag="ups", name=f"tp1{ci}")
                nc.tensor.transpose(tp1, w1nat[:, ci * P:(ci + 1) * P],
                                    ident[0:CR, 0:CR])
                nc.scalar.copy(w1T[:, ci, :], tp1)
                tp2 = upsp.tile([CR, P], f32, tag="ups", name=f"tp2{ci}")
                nc.tensor.transpose(tp2, w2nat[:, ci, :], ident)
                nc.scalar.copy(w2cat[0:CR, ci, :], tp2)
                nc.scalar.copy(w2cat[CR:P, ci, :], tp2)

            # x tiles keyed (b, ci, q); all 16 loaded up front (f16 halves
            # the footprint so both samples fit), q-major per sample so
            # arrival order matches consumption order.
            xtiles = {}

            def emit_x_load(bb, ci, q):
                t = xp.tile([P, QS, W], f16, tag="x", name=f"x{bb}{ci}{q}",
                            uniquify=True)
                xtiles[(bb, ci, q)] = t
                nc.gpsimd.dma_start(
                    out=t, in_=xv[bb, ci, :, q * QS:(q + 1) * QS, :])

            for q in range(NQ):
                for ci in range(CT):
                    emit_x_load(0, ci, q)

            # per-sample state
            st = {}

            def phase_a_open(b):
                mw = []; mhp = []; mh = []; pha = []; pwa = []
                avh_ps = []; avw_ps = []
                for ci in range(CT):
                    avh_ps.append(avhp.tile([P, W], f32, tag="avh",
                                            name=f"avh{b}{ci}"))
                    avw_ps.append(avwp.tile([P, H], f32, tag="avw",
                                            name=f"avw{b}{ci}"))
                for ci in range(CT):
                    mw.append(pl.tile([P, H], f32, tag="mw", name=f"mw{b}{ci}"))
                    mhp.append(pl.tile([P, NQ, W], f16, tag="mhp",
                                       name=f"mhp{b}{ci}"))
                    mh.append(pl.tile([P, W], f32, tag="mh", name=f"mh{b}{ci}"))
                    pha.append(pl.tile([P, W], f32, tag="pha",
                                       name=f"pha{b}{ci}"))
                    pwa.append(pl.tile([P, H], f32, tag="pwa",
                                       name=f"pwa{b}{ci}"))

                st[b] = dict(mw=mw, mhp=mhp, mh=mh, pha=pha, pwa=pwa,
                             avh_ps=avh_ps, avw_ps=avw_ps)

            def phase_a_tile(b, q, ci):
                s = st[b]
                t = xtiles[(b, ci, q)]
                mw, mhp = s["mw"], s["mhp"]
                # max over w: pairwise f16 TT tree (2x DVE mode) + short
                # 1x reduce over the last 16 columns
                sw = scw.tile([P, QS, W // 2], f16, tag="scw",
                              name=f"sw{b}{ci}{q}", uniquify=True)
                nc.vector.tensor_tensor(
                    out=sw, in0=t[:, :, 0:64], in1=t[:, :, 64:128],
                    op=Alu.max)
                nc.vector.tensor_tensor(
                    out=sw[:, :, 0:32], in0=sw[:, :, 0:32],
                    in1=sw[:, :, 32:64], op=Alu.max)
                nc.vector.tensor_tensor(
                    out=sw[:, :, 0:16], in0=sw[:, :, 0:16],
                    in1=sw[:, :, 16:32], op=Alu.max)
                nc.vector.tensor_reduce(
                    out=mw[ci][:, q * QS:(q + 1) * QS],
                    in_=sw[:, :, 0:16], axis=X, op=Alu.max)
                # partial max over h: f16 TT tree down to one h-row, all on
                # DVE (the Pool engine's software TensorTensor only handles
                # 4-byte dtypes, so it cannot read the f16 x tiles)
                sh = sch.tile([P, QS // 2, W], f16, tag="sch",
                              name=f"sh{b}{ci}{q}", uniquify=True)
                nc.vector.tensor_tensor(
                    out=sh, in0=t[:, 0:16, :], in1=t[:, 16:32, :],
                    op=Alu.max)
                nc.vector.tensor_tensor(
                    out=sh[:, 0:8, :], in0=sh[:, 0:8, :],
                    in1=sh[:, 8:16, :], op=Alu.max)
                nc.vector.tensor_tensor(
                    out=sh[:, 0:4, :], in0=sh[:, 0:4, :],
                    in1=sh[:, 4:8, :], op=Alu.max)
                nc.vector.tensor_tensor(
                    out=sh[:, 0:2, :], in0=sh[:, 0:2, :],
                    in1=sh[:, 2:4, :], op=Alu.max)
                nc.vector.tensor_tensor(
                    out=mhp[ci][:, q, :], in0=sh[:, 0, :], in1=sh[:, 1, :],
                    op=Alu.max)
                # mean over h (f16 PE, 4 h-rows per matmul into step-0 psum)
                avh_ps, avw_ps = s["avh_ps"], s["avw_ps"]
                for j in range(QS // 4):
                    nc.tensor.matmul(
                        step0_out(avh_ps[ci], 4, W),
                        ident16, t[:, 4 * j:4 * j + 4, :],
                        start=(q == 0 and j == 0),
                        stop=(q == NQ - 1 and j == QS // 4 - 1))
                # mean over w (f16 PE, 16 w-cols per matmul)
                for j in range(W // 16):
                    sl = avw_ps[ci][:, q * QS:(q + 1) * QS]
                    nc.tensor.matmul(
                        step0_out(sl, 16, QS),
                        ident16,
                        t[:, :, 16 * j:16 * j + 16].rearrange(
                            "p h w -> p w h"),
                        start=(j == 0), stop=(j == W // 16 - 1))
                if q < NQ - 1:
                    return
                # this c-tile fully pooled: combine pools
                nc.vector.tensor_reduce(
                    out=s["mh"][ci],
                    in_=mhp[ci].rearrange("p q w -> p w q"),
                    axis=X, op=Alu.max)
                nc.scalar.activation(out=s["pha"][ci], in_=avh_ps[ci],
                                     func=Act.Copy, bias=0.0,
                                     scale=1.0 / H)
                nc.scalar.activation(out=s["pwa"][ci], in_=avw_ps[ci],
                                     func=Act.Copy, bias=0.0,
                                     scale=1.0 / W)

            def phase_a_finish(b):
                s = st[b]
                u_ps = upsp.tile([P, 2, P], f32, tag="ups", name=f"ups{b}")
                for k, (rhs_a, rhs_m) in enumerate(
                        ((s["pha"], s["mh"]), (s["pwa"], s["mw"]))):
                    for ci in range(CT):
                        nc.tensor.matmul(
                            u_ps[0:CR, k, :], w1T[:, ci, :], rhs_a[ci],
                            start=(ci == 0), stop=(ci == CT - 1))
                    for ci in range(CT):
                        nc.tensor.matmul(
                            u_ps[CR:P, k, :], w1T[:, ci, :], rhs_m[ci],
                            start=(ci == 0), stop=(ci == CT - 1))
                u1s = pl.tile([P, W], f32, tag="u1s", name=f"u1s{b}")
                u2s = pl.tile([P, H], f32, tag="u2s", name=f"u2s{b}")
                nc.scalar.copy(u1s, u_ps[:, 0, :])
                nc.scalar.copy(u2s, u_ps[:, 1, :])
                st[b]["u1s"] = u1s
                st[b]["u2s"] = u2s

            def phase_c_stage1(b, blk):
                s = st[b]
                abi = gti.tile([P, BH, W], f32, tag="abi", name=f"abi{b}{blk}")
                abo = gto.tile([P, BH, W], f32r, tag="abo", name=f"abo{b}{blk}")
                u1b = bcast_ap(s["u1s"], BH, inner_last=True)
                u2sl = s["u2s"][:, blk * BH:(blk + 1) * BH]
                u2b = bcast_ap(u2sl, W, inner_last=False)
                nc.gpsimd.tensor_tensor(out=abi, in0=u1b, in1=u2b, op=Alu.add)
                # leaky relu into a separate tile (writes f32r): the bcast's
                # slot WAR then only trails the prelu, not the gate matmuls
                nc.scalar.activation(out=abo, in_=abi,
                                     func=Act.Prelu,
                                     bias=0.0, scale=1.0, alpha=NEG)
                st[(b, blk)] = abo

            def phase_c_stage2(b, blk):
                q = (blk * BH) // QS
                loc = blk * BH - q * QS
                abf = st.pop((b, blk)).rearrange("p h w -> p (h w)")
                for ci in range(CT):
                    sblk = sg.tile([P, BH * W], f32, tag="sig",
                                   name=f"s{b}{blk}{ci}")
                    for ss in range(NSUB):
                        pst = gpsp.tile([P, 512], f32, tag="gps",
                                        name=f"g{b}{blk}{ci}{ss}")
                        nc.tensor.matmul(
                            pst, w2cat[:, ci, :],
                            abf[:, 512 * ss:512 * (ss + 1)],
                            start=True, stop=True)
                        nc.scalar.activation(
                            out=sblk[:, 512 * ss:512 * (ss + 1)], in_=pst,
                            func=Act.Sigmoid, bias=0.0, scale=1.0)
                    xsl = xtiles[(b, ci, q)][:, loc:loc + BH, :].rearrange(
                        "p h w -> p (h w)")
                    # (s+1)*x written into the sigmoid tile
                    # (in0 aliasing out is safe)
                    nc.vector.scalar_tensor_tensor(
                        out=sblk, in0=sblk, scalar=1.0, in1=xsl,
                        op0=Alu.add, op1=Alu.mult)
                    nc.sync.dma_start(
                        out=ov[b, ci, :, blk * BH:(blk + 1) * BH, :],
                        in_=sblk)

            def phase_c(b):
                # two-stage software pipeline: bcast+prelu of block k+1 are
                # emitted BEFORE matmuls/sigmoids/STT of block k, so the
                # monotone engine-counter sem that guards the next block's
                # ab-slot WAR is reached without waiting for the previous
                # block's sigmoids
                for blk in range(NBLK + 1):
                    if blk < NBLK:
                        phase_c_stage1(b, blk)
                    if blk > 0:
                        phase_c_stage2(b, blk - 1)

            # ---- emission: phase-ordered with b0-gate / b1-tree
            # interleave on DVE ----
            # b0 trees run first; then b1's tree pairs alternate with b0's
            # gate stage-2 so the STTs (and the output stores behind them)
            # start ~30us earlier instead of draining after ALL b1 trees,
            # while b1's last tree still lands before Act finishes b0's
            # gate work (so u(b1) never stalls Act). Stage 1 (bcast+prelu)
            # runs two blocks ahead throughout.
            phase_a_open(0)
            for k in range(CT * NQ):
                phase_a_tile(0, k // CT, k % CT)
                # b1's loads issue during b0's pools: the Pool queue stays
                # short at the start and the DMA queue never goes idle
                emit_x_load(1, k % CT, k // CT)
            phase_a_finish(0)
            phase_a_open(1)
            phase_c_stage1(0, 0)
            phase_c_stage1(0, 1)
            for blk in range(NBLK // 2):
                for k in (2 * blk, 2 * blk + 1):
                    phase_a_tile(1, k // CT, k % CT)
                if blk + 2 < NBLK:
                    phase_c_stage1(0, blk + 2)
                phase_c_stage2(0, blk)
            for blk in range(NBLK // 2, NBLK):
                if blk + 2 < NBLK:
                    phase_c_stage1(0, blk + 2)
                phase_c_stage2(0, blk)
            phase_a_finish(1)
            phase_c(1)

    nc.finalize()
    return nc


def kernel(x, w1, w2):
    global _nc_cache
    if _nc_cache is None:
        _nc_cache = _build_nc()
    nc = _nc_cache

    from concourse.bass_utils import run_bass_kernel_spmd

    x = np.ascontiguousarray(np.asarray(x, dtype=np.float32))
    w1 = np.ascontiguousarray(np.asarray(w1, dtype=np.float32))
    w2 = np.ascontiguousarray(np.asarray(w2, dtype=np.float32))

    in_maps = [
        {"x": np.ascontiguousarray(x[i * BL:(i + 1) * BL]),
         "w1": w1, "w2": w2}
        for i in range(NCORES)
    ]
    res = run_bass_kernel_spmd(nc, in_maps, core_ids=list(range(NCORES)))
    return np.concatenate([r["out"] for r in res.results], axis=0)


# revision 40
# speedup vs baseline: 1.2371x; 1.2371x over previous
"""AxialChannelAttention TRN2 Bass kernel.

Full inputs: x [16,256,128,128] f32, w1 [64,256], w2 [256,64].
Sharding: data-parallel over batch, 2 samples per core on 8 cores.

Per-core dataflow (read-once/write-once HBM):
  - x loaded as 16 h-quarter tiles [128, 32, 128] via gpsimd SWDGE DMAs
    that cast f32->f16 on the fly: halves SBUF (both samples fully
    resident, no slot-recycling stalls) and keeps loads on a separate DMA
    queue from the output stores (no head-of-line blocking on the SP
    HWDGE queue). Loads are emitted 8 upfront + 8 during b0's pools so
    the default 1024-descriptor SWDGE ring never throttles (a bigger
    ring would eat per-partition SBUF).
  - max pools: pairwise tensor_tensor max trees on DVE (f16 2x_1p mode,
    0.52 ns/elem vs 1.04 for the 1x TensorReduce) with an in-place
    scratch tile per axis. All tree levels stay on DVE: the Pool
    engine's software TensorTensor handles 4-byte dtypes only, and
    TensorReduce/TensorScalarPtr have no DVE fast modes at all.
  - mean pools: PE identity-matmul in f16 with step-0 PSUM out APs
    accumulating 4 h-rows / 16 w-cols per matmul (f32 PSUM accumulate).
    The four accumulators keep separate PSUM banks: concurrent
    accumulation groups sharing one bank corrupt each other on HW.
  - u1s/u2s = w1 @ pools (PE, exact fp32, both branches on 128
    partitions), copied to SBUF on ACT (GPSIMD cannot touch PSUM).
  - gate per 2048-column block, software-pipelined in two stages:
    stage 1 = broadcast-add (GPSIMD TT, step-0 APs) into abi, leaky-relu
    (ACT Prelu) into a separate f32r abo tile; stage 2 = w2cat matmul
    (PE f32r), sigmoid (ACT from PSUM per 512), out = x*(1+s) (DVE
    scalar_tensor_tensor into the sigmoid tile, reading the f16 x),
    DMA out per block (SP HWDGE). Splitting abi/abo and emitting stage 1
    of block k+1 before stage 2 of block k keeps the monotone
    engine-counter semaphores that guard pool-slot reuse from chaining
    each block onto the previous block's sigmoids.
  - emission: b0 pools, then b1's tree pairs alternating with b0's gate
    stage-2 on DVE (one stage-2 block per two trees) so the STTs and the
    stores behind them start ~30us earlier than a pure phase ordering,
    while b1's last tree still lands before ACT finishes b0's gate work
    (u(b1) never stalls ACT). DVE ends up the near-saturated critical
    path (~154us busy of ~204us total). Separate scw/sch scratch tiles
    (not one shared tile) matter: sharing serializes the next tile's
    maxw tree behind the current tile's maxh and costs ~18us.

TimelineSim per-core: ~204.1us (baseline f32r kernel: 293.1us). Engine
busy: DVE ~154us (trees + final mult), ACT ~115us (prelu/sigmoid),
PE ~100us (mean pools + gate matmuls), GPSIMD ~84us (bcast + SWDGE
issue), DMA ~140us modeled / ~187us real-roofline (67MB at ~360GB/s).

f16 x introduces ~5e-4 relative rounding on the pools and the final
multiply; measured end-to-end relative error on HW: 7.1e-4 vs the fp32
reference (threshold 2e-2).
"""
import sys
import numpy as np

if "/opt/trn_rl_repo" not in sys.path:
    sys.path.insert(0, "/opt/trn_rl_repo")

B, C, H, W = 16, 256, 128, 128
CR, P = 64, 128
NCORES = 8
BL = B // NCORES          # samples per core
NEG = 0.01                # leaky relu slope
CT = C // P               # 2 c-tiles
NQ = 4                    # h-quarter tiles per (sample, c-tile)
QS = H // NQ              # 32 h-rows per x tile
NBLK = 8                  # gate blocks per sample (16 h-rows each)
BH = H // NBLK            # 16
NSUB = (BH * W) // 512    # 4 psum sub-blocks per gate block

_nc_cache = None


def _build_nc():
    import concourse.bacc as bacc
    import concourse.bass as bass
    import concourse.tile as tile
    from concourse import mybir
    from concourse.masks import make_identity

    f32 = mybir.dt.float32
    f32r = mybir.dt.float32r
    f16 = mybir.dt.float16
    Alu = mybir.AluOpType
    Act = mybir.ActivationFunctionType
    X = mybir.AxisListType.X

    # default 1024-descriptor SWDGE ring (the carveout eats per-partition
    # SBUF); loads are emitted 8 upfront + 8 interleaved so at most 8 are
    # ever in flight and the ring never throttles
    nc = bacc.Bacc()
    xd = nc.dram_tensor("x", [BL, C, H, W], f32, kind="ExternalInput")
    w1d = nc.dram_tensor("w1", [CR, C], f32, kind="ExternalInput")
    w2d = nc.dram_tensor("w2", [C, CR], f32, kind="ExternalInput")
    od = nc.dram_tensor("out", [BL, C, H, W], f32, kind="ExternalOutput")

    xv = xd[:].rearrange("b (ct cp) h w -> b ct cp h w", ct=CT)
    ov = od[:].rearrange("b (ct cp) h w -> b ct cp h w", ct=CT)

    def bcast_ap(t2d, n_rep, inner_last):
        if inner_last:
            return bass.AP(tensor=t2d.tensor, offset=t2d.offset,
                           ap=[list(t2d.ap[0]), [0, n_rep], list(t2d.ap[1])])
        return bass.AP(tensor=t2d.tensor, offset=t2d.offset,
                       ap=[list(t2d.ap[0]), list(t2d.ap[1]), [0, n_rep]])

    def step0_out(psl, n_rep, inner):
        return bass.AP(tensor=psl.tensor, offset=psl.offset,
                       ap=[list(psl.ap[0]), [0, n_rep], [1, inner]])

    with tile.TileContext(nc) as tc:
        with tc.tile_pool(name="const", bufs=1) as cst, \
             tc.tile_pool(name="xp", bufs=16) as xp, \
             tc.tile_pool(name="scw", bufs=1) as scw, \
             tc.tile_pool(name="sch", bufs=1) as sch, \
             tc.tile_pool(name="pool", bufs=3) as pl, \
             tc.tile_pool(name="gate_i", bufs=2) as gti, \
             tc.tile_pool(name="gate_o", bufs=2) as gto, \
             tc.tile_pool(name="sig", bufs=3) as sg, \
             tc.tile_pool(name="avh_ps", bufs=2, space="PSUM") as avhp, \
             tc.tile_pool(name="avw_ps", bufs=2, space="PSUM") as avwp, \
             tc.tile_pool(name="u_ps", bufs=1, space="PSUM") as upsp, \
             tc.tile_pool(name="g_ps", bufs=3, space="PSUM") as gpsp:

            ident = cst.tile([P, P], f32)
            make_identity(nc, ident)
            ident16 = cst.tile([P, P], f16)
            nc.scalar.copy(ident16, ident)
            # pre-warm the ACT function-table set (Copy/Prelu/Sigmoid):
            # the lazy LoadActFuncSet (~1.3us) otherwise lands in the first
            # gate block's critical chain
            warm = cst.tile([P, 4], f32)
            nc.scalar.activation(out=warm, in_=ident[:, 0:4],
                                 func=Act.Prelu, bias=0.0, scale=1.0,
                                 alpha=NEG)
            nc.scalar.activation(out=warm, in_=ident[:, 0:4],
                                 func=Act.Sigmoid, bias=0.0, scale=1.0)
            # weights: contiguous natural-layout DMAs + on-chip PE transpose
            # (strided 4-byte gather DMAs would cost ~3.6us each at the head
            # of the DMA queue)
            w1T = cst.tile([P, CT, CR], f32)
            w2cat = cst.tile([P, CT, P], f32r)
            w1nat = cst.tile([CR, C], f32)
            nc.sync.dma_start(out=w1nat, in_=w1d[:])
            w2nat = cst.tile([P, CT, CR], f32)
            w2vn = w2d[:].rearrange("(ct cp) r -> ct cp r", ct=CT)
            for ci in range(CT):
                nc.sync.dma_start(out=w2nat[:, ci, :], in_=w2vn[ci])
            for ci in range(CT):
                tp1 = upsp.tile([P, CR], f32, tag="ups", name=f"tp1{ci}")
                nc.tensor.transpose(tp1, w1nat[:, ci * P:(ci + 1) * P],
                                    ident[0:CR, 0:CR])
                nc.scalar.copy(w1T[:, ci, :], tp1)
                tp2 = upsp.tile([CR, P], f32, tag="ups", name=f"tp2{ci}")
                nc.tensor.transpose(tp2, w2nat[:, ci, :], ident)
                nc.scalar.copy(w2cat[0:CR, ci, :], tp2)
                nc.scalar.copy(w2cat[CR:P, ci, :], tp2)

            # x tiles keyed (b, ci, q); all 16 loaded up front (f16 halves
            # the footprint so both samples fit), q-major per sample so
            # arrival order matches consumption order.
            xtiles = {}

            def emit_x_load(bb, ci, q):
                t = xp.tile([P, QS, W], f16, tag="x", name=f"x{bb}{ci}{q}",
                            uniquify=True)
                xtiles[(bb, ci, q)] = t
                nc.gpsimd.dma_start(
                    out=t, in_=xv[bb, ci, :, q * QS:(q + 1) * QS, :])

            for q in range(NQ):
                for ci in range(CT):
                    emit_x_load(0, ci, q)

            # per-sample state
            st = {}

            def phase_a_open(b):
                mw = []; mhp = []; mh = []; pha = []; pwa = []
                avh_ps = []; avw_ps = []
                for ci in range(CT):
                    avh_ps.append(avhp.tile([P, W], f32, tag="avh",
                                            name=f"avh{b}{ci}"))
                    avw_ps.append(avwp.tile([P, H], f32, tag="avw",
                                            name=f"avw{b}{ci}"))
                for ci in range(CT):
                    mw.append(pl.tile([P, H], f32, tag="mw", name=f"mw{b}{ci}"))
                    mhp.append(pl.tile([P, NQ, W], f16, tag="mhp",
                                       name=f"mhp{b}{ci}"))
                    mh.append(pl.tile([P, W], f32, tag="mh", name=f"mh{b}{ci}"))
                    pha.append(pl.tile([P, W], f32, tag="pha",
                                       name=f"pha{b}{ci}"))
                    pwa.append(pl.tile([P, H], f32, tag="pwa",
                                       name=f"pwa{b}{ci}"))

                st[b] = dict(mw=mw, mhp=mhp, mh=mh, pha=pha, pwa=pwa,
                             avh_ps=avh_ps, avw_ps=avw_ps)

            def phase_a_tile(b, q, ci):
                s = st[b]
                t = xtiles[(b, ci, q)]
                mw, mhp = s["mw"], s["mhp"]
                # max over w: pairwise f16 TT tree (2x DVE mode) + short
                # 1x reduce over the last 16 columns
                sw = scw.tile([P, QS, W // 2], f16, tag="scw",
                              name=f"sw{b}{ci}{q}", uniquify=True)
                nc.vector.tensor_tensor(
                    out=sw, in0=t[:, :, 0:64], in1=t[:, :, 64:128],
                    op=Alu.max)
                nc.vector.tensor_tensor(
                    out=sw[:, :, 0:32], in0=sw[:, :, 0:32],
                    in1=sw[:, :, 32:64], op=Alu.max)
                nc.vector.tensor_tensor(
                    out=sw[:, :, 0:16], in0=sw[:, :, 0:16],
                    in1=sw[:, :, 16:32], op=Alu.max)
                nc.vector.tensor_reduce(
                    out=mw[ci][:, q * QS:(q + 1) * QS],
                    in_=sw[:, :, 0:16], axis=X, op=Alu.max)
                # partial max over h: f16 TT tree down to one h-row, all on
                # DVE (the Pool engine's software TensorTensor only handles
                # 4-byte dtypes, so it cannot read the f16 x tiles)
                sh = sch.tile([P, QS // 2, W], f16, tag="sch",
                              name=f"sh{b}{ci}{q}", uniquify=True)
                nc.vector.tensor_tensor(
                    out=sh, in0=t[:, 0:16, :], in1=t[:, 16:32, :],
                    op=Alu.max)
                nc.vector.tensor_tensor(
                    out=sh[:, 0:8, :], in0=sh[:, 0:8, :],
                    in1=sh[:, 8:16, :], op=Alu.max)
                nc.vector.tensor_tensor(
                    out=sh[:, 0:4, :], in0=sh[:, 0:4, :],
                    in1=sh[:, 4:8, :], op=Alu.max)
                nc.vector.tensor_tensor(
                    out=sh[:, 0:2, :], in0=sh[:, 0:2, :],
                    in1=sh[:, 2:4, :], op=Alu.max)
                nc.vector.tensor_tensor(
                    out=mhp[ci][:, q, :], in0=sh[:, 0, :], in1=sh[:, 1, :],
                    op=Alu.max)
                # mean over h (f16 PE, 4 h-rows per matmul into step-0 psum)
                avh_ps, avw_ps = s["avh_ps"], s["avw_ps"]
                for j in range(QS // 4):
                    nc.tensor.matmul(
                        step0_out(avh_ps[ci], 4, W),
                        ident16, t[:, 4 * j:4 * j + 4, :],
                        start=(q == 0 and j == 0),
                        stop=(q == NQ - 1 and j == QS // 4 - 1))
                # mean over w (f16 PE, 16 w-cols per matmul)
                for j in range(W // 16):
                    sl = avw_ps[ci][:, q * QS:(q + 1) * QS]
                    nc.tensor.matmul(
                        step0_out(sl, 16, QS),
                        ident16,
                        t[:, :, 16 * j:16 * j + 16].rearrange(
                            "p h w -> p w h"),
                        start=(j == 0), stop=(j == W // 16 - 1))
                if q < NQ - 1:
                    return
                # this c-tile fully pooled: combine pools
                nc.vector.tensor_reduce(
                    out=s["mh"][ci],
                    in_=mhp[ci].rearrange("p q w -> p w q"),
                    axis=X, op=Alu.max)
                nc.scalar.activation(out=s["pha"][ci], in_=avh_ps[ci],
                                     func=Act.Copy, bias=0.0,
                                     scale=1.0 / H)
                nc.scalar.activation(out=s["pwa"][ci], in_=avw_ps[ci],
                                     func=Act.Copy, bias=0.0,
                                     scale=1.0 / W)

            def phase_a_finish(b):
                s = st[b]
                u_ps = upsp.tile([P, 2, P], f32, tag="ups", name=f"ups{b}")
                for k, (rhs_a, rhs_m) in enumerate(
                        ((s["pha"], s["mh"]), (s["pwa"], s["mw"]))):
                    for ci in range(CT):
                        nc.tensor.matmul(
                            u_ps[0:CR, k, :], w1T[:, ci, :], rhs_a[ci],
                            start=(ci == 0), stop=(ci == CT - 1))
                    for ci in range(CT):
                        nc.tensor.matmul(
                            u_ps[CR:P, k, :], w1T[:, ci, :], rhs_m[ci],
                            start=(ci == 0), stop=(ci == CT - 1))
                u1s = pl.tile([P, W], f32, tag="u1s", name=f"u1s{b}")
                u2s = pl.tile([P, H], f32, tag="u2s", name=f"u2s{b}")
                nc.scalar.copy(u1s, u_ps[:, 0, :])
                nc.scalar.copy(u2s, u_ps[:, 1, :])
                st[b]["u1s"] = u1s
                st[b]["u2s"] = u2s

            def phase_c_stage1(b, blk):
                s = st[b]
                abi = gti.tile([P, BH, W], f32, tag="abi", name=f"abi{b}{blk}")
                abo = gto.tile([P, BH, W], f32r, tag="abo", name=f"abo{b}{blk}")
                u1b = bcast_ap(s["u1s"], BH, inner_last=True)
                u2sl = s["u2s"][:, blk * BH:(blk + 1) * BH]
                u2b = bcast_ap(u2sl, W, inner_last=False)
                nc.gpsimd.tensor_tensor(out=abi, in0=u1b, in1=u2b, op=Alu.add)
                # leaky relu into a separate tile (writes f32r): the bcast's
                # slot WAR then only trails the prelu, not the gate matmuls
                nc.scalar.activation(out=abo, in_=abi,
                                     func=Act.Prelu,
                                     bias=0.0, scale=1.0, alpha=NEG)
                st[(b, blk)] = abo

            def phase_c_stage2(b, blk):
                q = (blk * BH) // QS
                loc = blk * BH - q * QS
                abf = st.pop((b, blk)).rearrange("p h w -> p (h w)")
                for ci in range(CT):
                    sblk = sg.tile([P, BH * W], f32, tag="sig",
                                   name=f"s{b}{blk}{ci}")
                    for ss in range(NSUB):
                        pst = gpsp.tile([P, 512], f32, tag="gps",
                                        name=f"g{b}{blk}{ci}{ss}")
                        nc.tensor.matmul(
                            pst, w2cat[:, ci, :],
                            abf[:, 512 * ss:512 * (ss + 1)],
                            start=True, stop=True)
                        nc.scalar.activation(
                            out=sblk[:, 512 * ss:512 * (ss + 1)], in_=pst,
                            func=Act.Sigmoid, bias=0.0, scale=1.0)
                    xsl = xtiles[(b, ci, q)][:, loc:loc + BH, :].rearrange(
                        "p h w -> p (h w)")
                    # (s+1)*x written into the sigmoid tile
                    # (in0 aliasing out is safe)
                    nc.vector.scalar_tensor_tensor(
                        out=sblk, in0=sblk, scalar=1.0, in1=xsl,
                        op0=Alu.add, op1=Alu.mult)
                    nc.sync.dma_start(
                        out=ov[b, ci, :, blk * BH:(blk + 1) * BH, :],
                        in_=sblk)

            def phase_c(b):
                # two-stage software pipeline: bcast+prelu of block k+1 are
                # emitted BEFORE matmuls/sigmoids/STT of block k, so the
                # monotone engine-counter sem that guards the next block's
                # ab-slot WAR is reached without waiting for the previous
                # block's sigmoids
                for blk in range(NBLK + 1):
                    if blk < NBLK:
                        phase_c_stage1(b, blk)
                    if blk > 0:
                        phase_c_stage2(b, blk - 1)

            # ---- emission: phase-ordered with b0-gate / b1-tree
            # interleave on DVE ----
            # b0 trees run first; then b1's tree pairs alternate with b0's
            # gate stage-2 so the STTs (and the output stores behind them)
            # start ~30us earlier instead of draining after ALL b1 trees,
            # while b1's last tree still lands before Act finishes b0's
            # gate work (so u(b1) never stalls Act). Stage 1 (bcast+prelu)
            # runs two blocks ahead throughout.
            phase_a_open(0)
            for k in range(CT * NQ):
                phase_a_tile(0, k // CT, k % CT)
                # b1's loads issue during b0's pools: the Pool queue stays
                # short at the start and the DMA queue never goes idle
                emit_x_load(1, k % CT, k // CT)
            phase_a_finish(0)
            phase_a_open(1)
            phase_c_stage1(0, 0)
            phase_c_stage1(0, 1)
            for blk in range(NBLK // 2):
                for k in (2 * blk, 2 * blk + 1):
                    phase_a_tile(1, k // CT, k % CT)
                if blk + 2 < NBLK:
                    phase_c_stage1(0, blk + 2)
                phase_c_stage2(0, blk)
            for blk in range(NBLK // 2, NBLK):
                if blk + 2 < NBLK:
                    phase_c_stage1(0, blk + 2)
                phase_c_stage2(0, blk)
            phase_a_finish(1)
            phase_c(1)

    nc.finalize()
    return nc


def kernel(x, w1, w2):
    global _nc_cache
    if _nc_cache is None:
        _nc_cache = _build_nc()
    nc = _nc_cache

    from concourse.bass_utils import run_bass_kernel_spmd

    x = np.ascontiguousarray(np.asarray(x, dtype=np.float32))
    w1 = np.ascontiguousarray(np.asarray(w1, dtype=np.float32))
    w2 = np.ascontiguousarray(np.asarray(w2, dtype=np.float32))

    in_maps = [
        {"x": np.ascontiguousarray(x[i * BL:(i + 1) * BL]),
         "w1": w1, "w2": w2}
        for i in range(NCORES)
    ]
    res = run_bass_kernel_spmd(nc, in_maps, core_ids=list(range(NCORES)))
    return np.concatenate([r["out"] for r in res.results], axis=0)


# revision 42
# speedup vs baseline: 1.2534x; 1.0132x over previous
"""AxialChannelAttention TRN2 Bass kernel.

Full inputs: x [16,256,128,128] f32, w1 [64,256], w2 [256,64].
Sharding: data-parallel over batch, 2 samples per core on 8 cores.

Per-core dataflow (read-once/write-once HBM):
  - x loaded as 16 h-quarter tiles [128, 32, 128] via gpsimd SWDGE DMAs
    that cast f32->f16 on the fly: halves SBUF (both samples fully
    resident, no slot-recycling stalls) and keeps loads on a separate DMA
    queue from the output stores (no head-of-line blocking on the SP
    HWDGE queue). Loads are emitted 8 upfront + 8 during b0's pools so
    the default 1024-descriptor SWDGE ring never throttles (a bigger
    ring would eat per-partition SBUF).
  - max pools: pairwise tensor_tensor max trees on DVE (f16 2x_1p mode,
    0.52 ns/elem vs 1.04 for the 1x TensorReduce) with an in-place
    scratch tile per axis. All tree levels stay on DVE: the Pool
    engine's software TensorTensor handles 4-byte dtypes only, and
    TensorReduce/TensorScalarPtr have no DVE fast modes at all.
  - mean pools: PE identity-matmul in f16 with step-0 PSUM out APs
    accumulating 4 h-rows / 16 w-cols per matmul (f32 PSUM accumulate).
    The four accumulators keep separate PSUM banks: concurrent
    accumulation groups sharing one bank corrupt each other on HW.
  - u1s/u2s = w1 @ pools (PE, exact fp32, both branches on 128
    partitions), copied to SBUF on ACT (GPSIMD cannot touch PSUM).
  - gate per 2048-column block, software-pipelined in two stages:
    stage 1 = broadcast-add (GPSIMD TT, step-0 APs) into abi, leaky-relu
    (ACT Prelu) into a separate f32r abo tile; stage 2 = w2cat matmul
    (PE f32r), sigmoid (ACT from PSUM per 512), out = x*(1+s) (DVE
    scalar_tensor_tensor into the sigmoid tile, reading the f16 x),
    DMA out per block (SP HWDGE). Splitting abi/abo and emitting stage 1
    of block k+1 before stage 2 of block k keeps the monotone
    engine-counter semaphores that guard pool-slot reuse from chaining
    each block onto the previous block's sigmoids.
  - emission: b0 pools, then b1's tree pairs alternating with b0's gate
    stage-2 on DVE (one stage-2 block per two trees) so the STTs and the
    stores behind them start ~30us earlier than a pure phase ordering,
    while b1's last tree still lands before ACT finishes b0's gate work
    (u(b1) never stalls ACT). DVE ends up the near-saturated critical
    path (~154us busy of ~204us total). Separate scw/sch scratch tiles
    (not one shared tile) matter: sharing serializes the next tile's
    maxw tree behind the current tile's maxh and costs ~18us.

TimelineSim per-core: ~204.1us (baseline f32r kernel: 293.1us). Engine
busy: DVE ~154us (trees + final mult), ACT ~115us (prelu/sigmoid),
PE ~100us (mean pools + gate matmuls), GPSIMD ~84us (bcast + SWDGE
issue), DMA ~140us modeled / ~187us real-roofline (67MB at ~360GB/s).

f16 x introduces ~5e-4 relative rounding on the pools and the final
multiply; measured end-to-end relative error on HW: 7.1e-4 vs the fp32
reference (threshold 2e-2).
"""
import sys
import numpy as np

if "/opt/trn_rl_repo" not in sys.path:
    sys.path.insert(0, "/opt/trn_rl_repo")

B, C, H, W = 16, 256, 128, 128
CR, P = 64, 128
NCORES = 8
BL = B // NCORES          # samples per core
NEG = 0.01                # leaky relu slope
CT = C // P               # 2 c-tiles
NQ = 4                    # h-quarter tiles per (sample, c-tile)
QS = H // NQ              # 32 h-rows per x tile
NBLK = 8                  # gate blocks per sample (16 h-rows each)
BH = H // NBLK            # 16
NSUB = (BH * W) // 512    # 4 psum sub-blocks per gate block

_nc_cache = None


def _build_nc():
    import concourse.bacc as bacc
    import concourse.bass as bass
    import concourse.tile as tile
    from concourse import mybir
    from concourse.masks import make_identity

    f32 = mybir.dt.float32
    f32r = mybir.dt.float32r
    f16 = mybir.dt.float16
    Alu = mybir.AluOpType
    Act = mybir.ActivationFunctionType
    X = mybir.AxisListType.X

    # default 1024-descriptor SWDGE ring (the carveout eats per-partition
    # SBUF); loads are emitted 8 upfront + 8 interleaved so at most 8 are
    # ever in flight and the ring never throttles
    nc = bacc.Bacc()
    xd = nc.dram_tensor("x", [BL, C, H, W], f32, kind="ExternalInput")
    w1d = nc.dram_tensor("w1", [CR, C], f32, kind="ExternalInput")
    w2d = nc.dram_tensor("w2", [C, CR], f32, kind="ExternalInput")
    od = nc.dram_tensor("out", [BL, C, H, W], f32, kind="ExternalOutput")

    xv = xd[:].rearrange("b (ct cp) h w -> b ct cp h w", ct=CT)
    ov = od[:].rearrange("b (ct cp) h w -> b ct cp h w", ct=CT)

    def bcast_ap(t2d, n_rep, inner_last):
        if inner_last:
            return bass.AP(tensor=t2d.tensor, offset=t2d.offset,
                           ap=[list(t2d.ap[0]), [0, n_rep], list(t2d.ap[1])])
        return bass.AP(tensor=t2d.tensor, offset=t2d.offset,
                       ap=[list(t2d.ap[0]), list(t2d.ap[1]), [0, n_rep]])

    def step0_out(psl, n_rep, inner):
        return bass.AP(tensor=psl.tensor, offset=psl.offset,
                       ap=[list(psl.ap[0]), [0, n_rep], [1, inner]])

    with tile.TileContext(nc) as tc:
        with tc.tile_pool(name="const", bufs=1) as cst, \
             tc.tile_pool(name="xp", bufs=16) as xp, \
             tc.tile_pool(name="scw", bufs=1) as scw, \
             tc.tile_pool(name="sch", bufs=1) as sch, \
             tc.tile_pool(name="pool", bufs=3) as pl, \
             tc.tile_pool(name="gate_i", bufs=2) as gti, \
             tc.tile_pool(name="gate_o", bufs=2) as gto, \
             tc.tile_pool(name="sig", bufs=3) as sg, \
             tc.tile_pool(name="avh_ps", bufs=2, space="PSUM") as avhp, \
             tc.tile_pool(name="avw_ps", bufs=2, space="PSUM") as avwp, \
             tc.tile_pool(name="u_ps", bufs=1, space="PSUM") as upsp, \
             tc.tile_pool(name="g_ps", bufs=3, space="PSUM") as gpsp:

            ident = cst.tile([P, P], f32)
            make_identity(nc, ident)
            ident16 = cst.tile([P, P], f16)
            nc.scalar.copy(ident16, ident)
            # pre-warm the ACT function-table set (Copy/Prelu/Sigmoid):
            # the lazy LoadActFuncSet (~1.3us) otherwise lands in the first
            # gate block's critical chain
            warm = cst.tile([P, 4], f32)
            nc.scalar.activation(out=warm, in_=ident[:, 0:4],
                                 func=Act.Prelu, bias=0.0, scale=1.0,
                                 alpha=NEG)
            nc.scalar.activation(out=warm, in_=ident[:, 0:4],
                                 func=Act.Sigmoid, bias=0.0, scale=1.0)
            # weights: contiguous natural-layout DMAs + on-chip PE transpose
            # (strided 4-byte gather DMAs would cost ~3.6us each at the head
            # of the DMA queue)
            w1T = cst.tile([P, CT, CR], f32)
            w2cat = cst.tile([P, CT, P], f32r)
            w1nat = cst.tile([CR, C], f32)
            nc.sync.dma_start(out=w1nat, in_=w1d[:])
            w2nat = cst.tile([P, CT, CR], f32)
            w2vn = w2d[:].rearrange("(ct cp) r -> ct cp r", ct=CT)
            for ci in range(CT):
                nc.sync.dma_start(out=w2nat[:, ci, :], in_=w2vn[ci])
            for ci in range(CT):
                tp1 = upsp.tile([P, CR], f32, tag="ups", name=f"tp1{ci}")
                nc.tensor.transpose(tp1, w1nat[:, ci * P:(ci + 1) * P],
                                    ident[0:CR, 0:CR])
                nc.scalar.copy(w1T[:, ci, :], tp1)
                tp2 = upsp.tile([CR, P], f32, tag="ups", name=f"tp2{ci}")
                nc.tensor.transpose(tp2, w2nat[:, ci, :], ident)
                nc.scalar.copy(w2cat[0:CR, ci, :], tp2)
                nc.scalar.copy(w2cat[CR:P, ci, :], tp2)

            # x tiles keyed (b, ci, q); all 16 loaded up front (f16 halves
            # the footprint so both samples fit), q-major per sample so
            # arrival order matches consumption order.
            xtiles = {}

            def emit_x_load(bb, ci, q):
                t = xp.tile([P, QS, W], f16, tag="x", name=f"x{bb}{ci}{q}",
                            uniquify=True)
                xtiles[(bb, ci, q)] = t
                nc.gpsimd.dma_start(
                    out=t, in_=xv[bb, ci, :, q * QS:(q + 1) * QS, :])

            for q in range(NQ):
                for ci in range(CT):
                    emit_x_load(0, ci, q)

            # per-sample state
            st = {}

            def phase_a_open(b):
                mw = []; mhp = []; mh = []; pha = []; pwa = []
                avh_ps = []; avw_ps = []
                for ci in range(CT):
                    avh_ps.append(avhp.tile([P, W], f32, tag="avh",
                                            name=f"avh{b}{ci}"))
                    avw_ps.append(avwp.tile([P, H], f32, tag="avw",
                                            name=f"avw{b}{ci}"))
                for ci in range(CT):
                    mw.append(pl.tile([P, H], f32, tag="mw", name=f"mw{b}{ci}"))
                    mhp.append(pl.tile([P, NQ, W], f16, tag="mhp",
                                       name=f"mhp{b}{ci}"))
                    mh.append(pl.tile([P, W], f32, tag="mh", name=f"mh{b}{ci}"))
                    pha.append(pl.tile([P, W], f32, tag="pha",
                                       name=f"pha{b}{ci}"))
                    pwa.append(pl.tile([P, H], f32, tag="pwa",
                                       name=f"pwa{b}{ci}"))

                st[b] = dict(mw=mw, mhp=mhp, mh=mh, pha=pha, pwa=pwa,
                             avh_ps=avh_ps, avw_ps=avw_ps)

            def phase_a_tile(b, q, ci):
                s = st[b]
                t = xtiles[(b, ci, q)]
                mw, mhp = s["mw"], s["mhp"]
                # max over w: pairwise f16 TT tree (2x DVE mode) + short
                # 1x reduce over the last 16 columns
                sw = scw.tile([P, QS, W // 2], f16, tag="scw",
                              name=f"sw{b}{ci}{q}", uniquify=True)
                nc.vector.tensor_tensor(
                    out=sw, in0=t[:, :, 0:64], in1=t[:, :, 64:128],
                    op=Alu.max)
                nc.vector.tensor_tensor(
                    out=sw[:, :, 0:32], in0=sw[:, :, 0:32],
                    in1=sw[:, :, 32:64], op=Alu.max)
                nc.vector.tensor_tensor(
                    out=sw[:, :, 0:16], in0=sw[:, :, 0:16],
                    in1=sw[:, :, 16:32], op=Alu.max)
                nc.vector.tensor_reduce(
                    out=mw[ci][:, q * QS:(q + 1) * QS],
                    in_=sw[:, :, 0:16], axis=X, op=Alu.max)
                # partial max over h: f16 TT tree down to one h-row, all on
                # DVE (the Pool engine's software TensorTensor only handles
                # 4-byte dtypes, so it cannot read the f16 x tiles)
                sh = sch.tile([P, QS // 2, W], f16, tag="sch",
                              name=f"sh{b}{ci}{q}", uniquify=True)
                nc.vector.tensor_tensor(
                    out=sh, in0=t[:, 0:16, :], in1=t[:, 16:32, :],
                    op=Alu.max)
                nc.vector.tensor_tensor(
                    out=sh[:, 0:8, :], in0=sh[:, 0:8, :],
                    in1=sh[:, 8:16, :], op=Alu.max)
                nc.vector.tensor_tensor(
                    out=sh[:, 0:4, :], in0=sh[:, 0:4, :],
                    in1=sh[:, 4:8, :], op=Alu.max)
                nc.vector.tensor_tensor(
                    out=sh[:, 0:2, :], in0=sh[:, 0:2, :],
                    in1=sh[:, 2:4, :], op=Alu.max)
                nc.vector.tensor_tensor(
                    out=mhp[ci][:, q, :], in0=sh[:, 0, :], in1=sh[:, 1, :],
                    op=Alu.max)
                # mean over h (f16 PE, 4 h-rows per matmul into step-0 psum)
                avh_ps, avw_ps = s["avh_ps"], s["avw_ps"]
                for j in range(QS // 4):
                    nc.tensor.matmul(
                        step0_out(avh_ps[ci], 4, W),
                        ident16, t[:, 4 * j:4 * j + 4, :],
                        start=(q == 0 and j == 0),
                        stop=(q == NQ - 1 and j == QS // 4 - 1))
                # mean over w (f16 PE, 16 w-cols per matmul)
                for j in range(W // 16):
                    sl = avw_ps[ci][:, q * QS:(q + 1) * QS]
                    nc.tensor.matmul(
                        step0_out(sl, 16, QS),
                        ident16,
                        t[:, :, 16 * j:16 * j + 16].rearrange(
                            "p h w -> p w h"),
                        start=(j == 0), stop=(j == W // 16 - 1))
                if q < NQ - 1:
                    return
                # this c-tile fully pooled: combine pools
                nc.vector.tensor_reduce(
                    out=s["mh"][ci],
                    in_=mhp[ci].rearrange("p q w -> p w q"),
                    axis=X, op=Alu.max)
                nc.scalar.activation(out=s["pha"][ci], in_=avh_ps[ci],
                                     func=Act.Copy, bias=0.0,
                                     scale=1.0 / H)
                nc.scalar.activation(out=s["pwa"][ci], in_=avw_ps[ci],
                                     func=Act.Copy, bias=0.0,
                                     scale=1.0 / W)

            def phase_a_finish(b):
                s = st[b]
                u_ps = upsp.tile([P, 2, P], f32, tag="ups", name=f"ups{b}")
                for k, (rhs_a, rhs_m) in enumerate(
                        ((s["pha"], s["mh"]), (s["pwa"], s["mw"]))):
                    for ci in range(CT):
                        nc.tensor.matmul(
                            u_ps[0:CR, k, :], w1T[:, ci, :], rhs_a[ci],
                            start=(ci == 0), stop=(ci == CT - 1))
                    for ci in range(CT):
                        nc.tensor.matmul(
                            u_ps[CR:P, k, :], w1T[:, ci, :], rhs_m[ci],
                            start=(ci == 0), stop=(ci == CT - 1))
                u1s = pl.tile([P, W], f32, tag="u1s", name=f"u1s{b}")
                u2s = pl.tile([P, H], f32, tag="u2s", name=f"u2s{b}")
                nc.scalar.copy(u1s, u_ps[:, 0, :])
                nc.scalar.copy(u2s, u_ps[:, 1, :])
                st[b]["u1s"] = u1s
                st[b]["u2s"] = u2s

            def phase_c_stage1(b, blk):
                s = st[b]
                abi = gti.tile([P, BH, W], f32, tag="abi", name=f"abi{b}{blk}")
                abo = gto.tile([P, BH, W], f32r, tag="abo", name=f"abo{b}{blk}")
                u1b = bcast_ap(s["u1s"], BH, inner_last=True)
                u2sl = s["u2s"][:, blk * BH:(blk + 1) * BH]
                u2b = bcast_ap(u2sl, W, inner_last=False)
                nc.gpsimd.tensor_tensor(out=abi, in0=u1b, in1=u2b, op=Alu.add)
                # leaky relu into a separate tile (writes f32r): the bcast's
                # slot WAR then only trails the prelu, not the gate matmuls
                nc.scalar.activation(out=abo, in_=abi,
                                     func=Act.Prelu,
                                     bias=0.0, scale=1.0, alpha=NEG)
                st[(b, blk)] = abo

            def phase_c_stage2(b, blk):
                q = (blk * BH) // QS
                loc = blk * BH - q * QS
                abf = st.pop((b, blk)).rearrange("p h w -> p (h w)")
                for ci in range(CT):
                    sblk = sg.tile([P, BH * W], f32, tag="sig",
                                   name=f"s{b}{blk}{ci}")
                    for ss in range(NSUB):
                        pst = gpsp.tile([P, 512], f32, tag="gps",
                                        name=f"g{b}{blk}{ci}{ss}")
                        nc.tensor.matmul(
                            pst, w2cat[:, ci, :],
                            abf[:, 512 * ss:512 * (ss + 1)],
                            start=True, stop=True)
                        nc.scalar.activation(
                            out=sblk[:, 512 * ss:512 * (ss + 1)], in_=pst,
                            func=Act.Sigmoid, bias=0.0, scale=1.0)
                    xsl = xtiles[(b, ci, q)][:, loc:loc + BH, :].rearrange(
                        "p h w -> p (h w)")
                    # (s+1)*x written into the sigmoid tile
                    # (in0 aliasing out is safe)
                    nc.vector.scalar_tensor_tensor(
                        out=sblk, in0=sblk, scalar=1.0, in1=xsl,
                        op0=Alu.add, op1=Alu.mult)
                    nc.sync.dma_start(
                        out=ov[b, ci, :, blk * BH:(blk + 1) * BH, :],
                        in_=sblk)

            def phase_c(b):
                # two-stage software pipeline: bcast+prelu of block k+1 are
                # emitted BEFORE matmuls/sigmoids/STT of block k, so the
                # monotone engine-counter sem that guards the next block's
                # ab-slot WAR is reached without waiting for the previous
                # block's sigmoids
                for blk in range(NBLK + 1):
                    if blk < NBLK:
                        phase_c_stage1(b, blk)
                    if blk > 0:
                        phase_c_stage2(b, blk - 1)

            # ---- emission: phase-ordered with b0-gate / b1-tree
            # interleave on DVE ----
            # b0 trees run first; then b1's tree pairs alternate with b0's
            # gate stage-2 so the STTs (and the output stores behind them)
            # start ~30us earlier instead of draining after ALL b1 trees,
            # while b1's last tree still lands before Act finishes b0's
            # gate work (so u(b1) never stalls Act). Stage 1 (bcast+prelu)
            # runs two blocks ahead throughout.
            phase_a_open(0)
            for k in range(CT * NQ):
                phase_a_tile(0, k // CT, k % CT)
                # b1's loads issue during b0's pools: the Pool queue stays
                # short at the start and the DMA queue never goes idle
                emit_x_load(1, k % CT, k // CT)
            phase_a_finish(0)
            phase_a_open(1)
            phase_c_stage1(0, 0)
            phase_c_stage1(0, 1)
            for blk in range(NBLK // 2):
                for k in (2 * blk, 2 * blk + 1):
                    phase_a_tile(1, k // CT, k % CT)
                if blk + 2 < NBLK:
                    phase_c_stage1(0, blk + 2)
                phase_c_stage2(0, blk)
            for blk in range(NBLK // 2, NBLK):
                if blk + 2 < NBLK:
                    phase_c_stage1(0, blk + 2)
                phase_c_stage2(0, blk)
            phase_a_finish(1)
            phase_c(1)

    nc.finalize()
    return nc


def kernel(x, w1, w2):
    global _nc_cache
    if _nc_cache is None:
        _nc_cache = _build_nc()
    nc = _nc_cache

    from concourse.bass_utils import run_bass_kernel_spmd

    x = np.ascontiguousarray(np.asarray(x, dtype=np.float32))
    w1 = np.ascontiguousarray(np.asarray(w1, dtype=np.float32))
    w2 = np.ascontiguousarray(np.asarray(w2, dtype=np.float32))

    in_maps = [
        {"x": np.ascontiguousarray(x[i * BL:(i + 1) * BL]),
         "w1": w1, "w2": w2}
        for i in range(NCORES)
    ]
    res = run_bass_kernel_spmd(nc, in_maps, core_ids=list(range(NCORES)))
    return np.concatenate([r["out"] for r in res.results], axis=0)


# revision 44
# speedup vs baseline: 1.9368x; 1.5452x over previous
"""AxialChannelAttention TRN2 Bass kernel.

Full inputs: x [16,256,128,128] f32, w1 [64,256], w2 [256,64].
Sharding: data-parallel over batch, 2 samples per core on 8 cores.

Per-core dataflow (read-once/write-once HBM):
  - x loaded as 16 h-quarter tiles [128, 32, 128] via gpsimd SWDGE DMAs
    that cast f32->f16 on the fly: halves SBUF (both samples fully
    resident, no slot-recycling stalls) and keeps loads on a separate DMA
    queue from the output stores (no head-of-line blocking on the SP
    HWDGE queue). Loads are emitted 8 upfront + 8 during b0's pools so
    the default 1024-descriptor SWDGE ring never throttles (a bigger
    ring would eat per-partition SBUF).
  - max pools: pairwise tensor_tensor max trees on DVE (f16 2x_1p mode,
    0.52 ns/elem vs 1.04 for the 1x TensorReduce) with an in-place
    scratch tile per axis. All tree levels stay on DVE: the Pool
    engine's software TensorTensor handles 4-byte dtypes only, and
    TensorReduce/TensorScalarPtr have no DVE fast modes at all.
  - mean pools: PE identity-matmul in f16 with step-0 PSUM out APs
    accumulating 4 h-rows / 16 w-cols per matmul (f32 PSUM accumulate).
    The four accumulators keep separate PSUM banks: concurrent
    accumulation groups sharing one bank corrupt each other on HW.
  - u1s/u2s = w1 @ pools (PE, exact fp32, both branches on 128
    partitions), copied to SBUF on ACT (GPSIMD cannot touch PSUM).
  - gate per 2048-column block, software-pipelined in two stages:
    stage 1 = broadcast-add (GPSIMD TT, step-0 APs) into abi, leaky-relu
    (ACT Prelu) into a separate f32r abo tile; stage 2 = w2cat matmul
    (PE f32r), sigmoid (ACT from PSUM per 512), out = x*(1+s) (DVE
    scalar_tensor_tensor into the sigmoid tile, reading the f16 x),
    DMA out per block (SP HWDGE). Splitting abi/abo and emitting stage 1
    of block k+1 before stage 2 of block k keeps the monotone
    engine-counter semaphores that guard pool-slot reuse from chaining
    each block onto the previous block's sigmoids.
  - emission: b0 pools, then b1's tree pairs alternating with b0's gate
    stage-2 on DVE (one stage-2 block per two trees) so the STTs and the
    stores behind them start ~30us earlier than a pure phase ordering,
    while b1's last tree still lands before ACT finishes b0's gate work
    (u(b1) never stalls ACT). DVE ends up the near-saturated critical
    path (~154us busy of ~204us total). Separate scw/sch scratch tiles
    (not one shared tile) matter: sharing serializes the next tile's
    maxw tree behind the current tile's maxh and costs ~18us.

TimelineSim per-core: ~204.1us (baseline f32r kernel: 293.1us). Engine
busy: DVE ~154us (trees + final mult), ACT ~115us (prelu/sigmoid),
PE ~100us (mean pools + gate matmuls), GPSIMD ~84us (bcast + SWDGE
issue), DMA ~140us modeled / ~187us real-roofline (67MB at ~360GB/s).

f16 x introduces ~5e-4 relative rounding on the pools and the final
multiply; measured end-to-end relative error on HW: 7.1e-4 vs the fp32
reference (threshold 2e-2).
"""
import sys
import numpy as np

if "/opt/trn_rl_repo" not in sys.path:
    sys.path.insert(0, "/opt/trn_rl_repo")

B, C, H, W = 16, 256, 128, 128
CR, P = 64, 128
NCORES = 8
BL = B // NCORES          # samples per core
NEG = 0.01                # leaky relu slope
CT = C // P               # 2 c-tiles
NQ = 4                    # h-quarter tiles per (sample, c-tile)
QS = H // NQ              # 32 h-rows per x tile
NBLK = 8                  # gate blocks per sample (16 h-rows each)
BH = H // NBLK            # 16
NSUB = (BH * W) // 512    # 4 psum sub-blocks per gate block

_nc_cache = None


def _build_nc():
    import concourse.bacc as bacc
    import concourse.bass as bass
    import concourse.tile as tile
    from concourse import mybir
    from concourse.masks import make_identity

    f32 = mybir.dt.float32
    f32r = mybir.dt.float32r
    f16 = mybir.dt.float16
    Alu = mybir.AluOpType
    Act = mybir.ActivationFunctionType
    X = mybir.AxisListType.X

    # default 1024-descriptor SWDGE ring (the carveout eats per-partition
    # SBUF); loads are emitted 8 upfront + 8 interleaved so at most 8 are
    # ever in flight and the ring never throttles
    nc = bacc.Bacc()
    xd = nc.dram_tensor("x", [BL, C, H, W], f32, kind="ExternalInput")
    w1d = nc.dram_tensor("w1", [CR, C], f32, kind="ExternalInput")
    w2d = nc.dram_tensor("w2", [C, CR], f32, kind="ExternalInput")
    od = nc.dram_tensor("out", [BL, C, H, W], f32, kind="ExternalOutput")

    xv = xd[:].rearrange("b (ct cp) h w -> b ct cp h w", ct=CT)
    ov = od[:].rearrange("b (ct cp) h w -> b ct cp h w", ct=CT)

    def bcast_ap(t2d, n_rep, inner_last):
        if inner_last:
            return bass.AP(tensor=t2d.tensor, offset=t2d.offset,
                           ap=[list(t2d.ap[0]), [0, n_rep], list(t2d.ap[1])])
        return bass.AP(tensor=t2d.tensor, offset=t2d.offset,
                       ap=[list(t2d.ap[0]), list(t2d.ap[1]), [0, n_rep]])

    def step0_out(psl, n_rep, inner):
        return bass.AP(tensor=psl.tensor, offset=psl.offset,
                       ap=[list(psl.ap[0]), [0, n_rep], [1, inner]])

    with tile.TileContext(nc) as tc:
        with tc.tile_pool(name="const", bufs=1) as cst, \
             tc.tile_pool(name="xp", bufs=16) as xp, \
             tc.tile_pool(name="scw", bufs=1) as scw, \
             tc.tile_pool(name="sch", bufs=1) as sch, \
             tc.tile_pool(name="pool", bufs=3) as pl, \
             tc.tile_pool(name="gate_i", bufs=2) as gti, \
             tc.tile_pool(name="gate_o", bufs=2) as gto, \
             tc.tile_pool(name="sig", bufs=3) as sg, \
             tc.tile_pool(name="avh_ps", bufs=1, space="PSUM") as avhp, \
             tc.tile_pool(name="avw_ps", bufs=1, space="PSUM") as avwp, \
             tc.tile_pool(name="u_ps", bufs=1, space="PSUM") as upsp, \
             tc.tile_pool(name="g_ps", bufs=2, space="PSUM") as gpsp:

            ident = cst.tile([P, P], f32)
            make_identity(nc, ident)
            ident16 = cst.tile([P, P], f16)
            nc.scalar.copy(ident16, ident)
            # pre-warm the ACT function-table set (Copy/Prelu/Sigmoid):
            # the lazy LoadActFuncSet (~1.3us) otherwise lands in the first
            # gate block's critical chain
            warm = cst.tile([P, 4], f32)
            nc.scalar.activation(out=warm, in_=ident[:, 0:4],
                                 func=Act.Prelu, bias=0.0, scale=1.0,
                                 alpha=NEG)
            nc.scalar.activation(out=warm, in_=ident[:, 0:4],
                                 func=Act.Sigmoid, bias=0.0, scale=1.0)
            # weights: contiguous natural-layout DMAs + on-chip PE transpose
            # (strided 4-byte gather DMAs would cost ~3.6us each at the head
            # of the DMA queue)
            w1T = cst.tile([P, CT, CR], f32)
            w2cat = cst.tile([P, CT, P], f32r)
            w1nat = cst.tile([CR, C], f32)
            nc.sync.dma_start(out=w1nat, in_=w1d[:])
            w2nat = cst.tile([P, CT, CR], f32)
            w2vn = w2d[:].rearrange("(ct cp) r -> ct cp r", ct=CT)
            for ci in range(CT):
                nc.sync.dma_start(out=w2nat[:, ci, :], in_=w2vn[ci])
            for ci in range(CT):
                tp1 = upsp.tile([P, CR], f32, tag="ups", name=f"tp1{ci}")
                nc.tensor.transpose(tp1, w1nat[:, ci * P:(ci + 1) * P],
                                    ident[0:CR, 0:CR])
                nc.scalar.copy(w1T[:, ci, :], tp1)
                tp2 = upsp.tile([CR, P], f32, tag="ups", name=f"tp2{ci}")
                nc.tensor.transpose(tp2, w2nat[:, ci, :], ident)
                nc.scalar.copy(w2cat[0:CR, ci, :], tp2)
                nc.scalar.copy(w2cat[CR:P, ci, :], tp2)

            # x tiles keyed (b, ci, q); all 16 loaded up front (f16 halves
            # the footprint so both samples fit), q-major per sample so
            # arrival order matches consumption order.
            xtiles = {}

            def emit_x_load(bb, ci, q):
                t = xp.tile([P, QS, W], f16, tag="x", name=f"x{bb}{ci}{q}",
                            uniquify=True)
                xtiles[(bb, ci, q)] = t
                nc.gpsimd.dma_start(
                    out=t, in_=xv[bb, ci, :, q * QS:(q + 1) * QS, :])

            for q in range(NQ):
                for ci in range(CT):
                    emit_x_load(0, ci, q)

            # per-sample state
            st = {}

            def phase_a_open(b):
                mw = []; mhp = []; mh = []; pha = []; pwa = []
                avh_ps = []; avw_ps = []
                for ci in range(CT):
                    avh_ps.append(avhp.tile([P, W], f32, tag="avh",
                                            name=f"avh{b}{ci}"))
                    avw_ps.append(avwp.tile([P, H], f32, tag="avw",
                                            name=f"avw{b}{ci}"))
                for ci in range(CT):
                    mw.append(pl.tile([P, H], f32, tag="mw", name=f"mw{b}{ci}"))
                    mhp.append(pl.tile([P, NQ, W], f16, tag="mhp",
                                       name=f"mhp{b}{ci}"))
                    mh.append(pl.tile([P, W], f32, tag="mh", name=f"mh{b}{ci}"))
                    pha.append(pl.tile([P, W], f32, tag="pha",
                                       name=f"pha{b}{ci}"))
                    pwa.append(pl.tile([P, H], f32, tag="pwa",
                                       name=f"pwa{b}{ci}"))

                st[b] = dict(mw=mw, mhp=mhp, mh=mh, pha=pha, pwa=pwa,
                             avh_ps=avh_ps, avw_ps=avw_ps)

            def phase_a_tile(b, q, ci):
                s = st[b]
                t = xtiles[(b, ci, q)]
                mw, mhp = s["mw"], s["mhp"]
                # max over w: pairwise f16 TT tree (2x DVE mode) + short
                # 1x reduce over the last 16 columns
                sw = scw.tile([P, QS, W // 2], f16, tag="scw",
                              name=f"sw{b}{ci}{q}", uniquify=True)
                nc.vector.tensor_tensor(
                    out=sw, in0=t[:, :, 0:64], in1=t[:, :, 64:128],
                    op=Alu.max)
                nc.vector.tensor_tensor(
                    out=sw[:, :, 0:32], in0=sw[:, :, 0:32],
                    in1=sw[:, :, 32:64], op=Alu.max)
                nc.vector.tensor_tensor(
                    out=sw[:, :, 0:16], in0=sw[:, :, 0:16],
                    in1=sw[:, :, 16:32], op=Alu.max)
                nc.vector.tensor_reduce(
                    out=mw[ci][:, q * QS:(q + 1) * QS],
                    in_=sw[:, :, 0:16], axis=X, op=Alu.max)
                # partial max over h: f16 TT tree down to one h-row, all on
                # DVE (the Pool engine's software TensorTensor only handles
                # 4-byte dtypes, so it cannot read the f16 x tiles)
                sh = sch.tile([P, QS // 2, W], f16, tag="sch",
                              name=f"sh{b}{ci}{q}", uniquify=True)
                nc.vector.tensor_tensor(
                    out=sh, in0=t[:, 0:16, :], in1=t[:, 16:32, :],
                    op=Alu.max)
                nc.vector.tensor_tensor(
                    out=sh[:, 0:8, :], in0=sh[:, 0:8, :],
                    in1=sh[:, 8:16, :], op=Alu.max)
                nc.vector.tensor_tensor(
                    out=sh[:, 0:4, :], in0=sh[:, 0:4, :],
                    in1=sh[:, 4:8, :], op=Alu.max)
                nc.vector.tensor_tensor(
                    out=sh[:, 0:2, :], in0=sh[:, 0:2, :],
                    in1=sh[:, 2:4, :], op=Alu.max)
                nc.vector.tensor_tensor(
                    out=mhp[ci][:, q, :], in0=sh[:, 0, :], in1=sh[:, 1, :],
                    op=Alu.max)
                # mean over h (f16 PE, 4 h-rows per matmul into step-0 psum)
                avh_ps, avw_ps = s["avh_ps"], s["avw_ps"]
                for j in range(QS // 4):
                    nc.tensor.matmul(
                        step0_out(avh_ps[ci], 4, W),
                        ident16, t[:, 4 * j:4 * j + 4, :],
                        start=(q == 0 and j == 0),
                        stop=(q == NQ - 1 and j == QS // 4 - 1))
                # mean over w (f16 PE, 16 w-cols per matmul)
                for j in range(W // 16):
                    sl = avw_ps[ci][:, q * QS:(q + 1) * QS]
                    nc.tensor.matmul(
                        step0_out(sl, 16, QS),
                        ident16,
                        t[:, :, 16 * j:16 * j + 16].rearrange(
                            "p h w -> p w h"),
                        start=(j == 0), stop=(j == W // 16 - 1))
                if q < NQ - 1:
                    return
                # this c-tile fully pooled: combine pools
                nc.vector.tensor_reduce(
                    out=s["mh"][ci],
                    in_=mhp[ci].rearrange("p q w -> p w q"),
                    axis=X, op=Alu.max)
                nc.scalar.activation(out=s["pha"][ci], in_=avh_ps[ci],
                                     func=Act.Copy, bias=0.0,
                                     scale=1.0 / H)
                nc.scalar.activation(out=s["pwa"][ci], in_=avw_ps[ci],
                                     func=Act.Copy, bias=0.0,
                                     scale=1.0 / W)

            def phase_a_finish(b):
                s = st[b]
                u_ps = upsp.tile([P, 2, P], f32, tag="ups", name=f"ups{b}")
                for k, (rhs_a, rhs_m) in enumerate(
                        ((s["pha"], s["mh"]), (s["pwa"], s["mw"]))):
                    for ci in range(CT):
                        nc.tensor.matmul(
                            u_ps[0:CR, k, :], w1T[:, ci, :], rhs_a[ci],
                            start=(ci == 0), stop=(ci == CT - 1))
                    for ci in range(CT):
                        nc.tensor.matmul(
                            u_ps[CR:P, k, :], w1T[:, ci, :], rhs_m[ci],
                            start=(ci == 0), stop=(ci == CT - 1))
                u1s = pl.tile([P, W], f32, tag="u1s", name=f"u1s{b}")
                u2s = pl.tile([P, H], f32, tag="u2s", name=f"u2s{b}")
                nc.scalar.copy(u1s, u_ps[:, 0, :])
                nc.scalar.copy(u2s, u_ps[:, 1, :])
                st[b]["u1s"] = u1s
                st[b]["u2s"] = u2s

            def phase_c_stage1(b, blk):
                s = st[b]
                abi = gti.tile([P, BH, W], f32, tag="abi", name=f"abi{b}{blk}")
                abo = gto.tile([P, BH, W], f32r, tag="abo", name=f"abo{b}{blk}")
                u1b = bcast_ap(s["u1s"], BH, inner_last=True)
                u2sl = s["u2s"][:, blk * BH:(blk + 1) * BH]
                u2b = bcast_ap(u2sl, W, inner_last=False)
                nc.gpsimd.tensor_tensor(out=abi, in0=u1b, in1=u2b, op=Alu.add)
                # leaky relu into a separate tile (writes f32r): the bcast's
                # slot WAR then only trails the prelu, not the gate matmuls
                nc.scalar.activation(out=abo, in_=abi,
                                     func=Act.Prelu,
                                     bias=0.0, scale=1.0, alpha=NEG)
                st[(b, blk)] = abo

            def phase_c_stage2(b, blk):
                q = (blk * BH) // QS
                loc = blk * BH - q * QS
                abf = st.pop((b, blk)).rearrange("p h w -> p (h w)")
                for ci in range(CT):
                    sblk = sg.tile([P, BH * W], f32, tag="sig",
                                   name=f"s{b}{blk}{ci}")
                    for hh in range(NSUB // 2):
                        # [P,1024] psum = 2 banks; each matmul is a
                        # self-contained start/stop group writing exactly
                        # one bank (the cross-group corruption hazard is
                        # only for groups SHARING a bank)
                        pst = gpsp.tile([P, 1024], f32, tag="gps",
                                        name=f"g{b}{blk}{ci}{hh}")
                        for ss in range(2):
                            nc.tensor.matmul(
                                pst[:, 512 * ss:512 * (ss + 1)],
                                w2cat[:, ci, :],
                                abf[:, 1024 * hh + 512 * ss:
                                    1024 * hh + 512 * (ss + 1)],
                                start=True, stop=True)
                        nc.scalar.activation(
                            out=sblk[:, 1024 * hh:1024 * (hh + 1)], in_=pst,
                            func=Act.Sigmoid, bias=0.0, scale=1.0)
                    xsl = xtiles[(b, ci, q)][:, loc:loc + BH, :].rearrange(
                        "p h w -> p (h w)")
                    # (s+1)*x written into the sigmoid tile
                    # (in0 aliasing out is safe)
                    nc.vector.scalar_tensor_tensor(
                        out=sblk, in0=sblk, scalar=1.0, in1=xsl,
                        op0=Alu.add, op1=Alu.mult)
                    nc.sync.dma_start(
                        out=ov[b, ci, :, blk * BH:(blk + 1) * BH, :],
                        in_=sblk)

            def phase_c(b):
                # two-stage software pipeline: bcast+prelu of block k+1 are
                # emitted BEFORE matmuls/sigmoids/STT of block k, so the
                # monotone engine-counter sem that guards the next block's
                # ab-slot WAR is reached without waiting for the previous
                # block's sigmoids
                for blk in range(NBLK + 1):
                    if blk < NBLK:
                        phase_c_stage1(b, blk)
                    if blk > 0:
                        phase_c_stage2(b, blk - 1)

            # ---- emission: phase-ordered with b0-gate / b1-tree
            # interleave on DVE ----
            # b0 trees run first; then b1's tree pairs alternate with b0's
            # gate stage-2 so the STTs (and the output stores behind them)
            # start ~30us earlier instead of draining after ALL b1 trees,
            # while b1's last tree still lands before Act finishes b0's
            # gate work (so u(b1) never stalls Act). Stage 1 (bcast+prelu)
            # runs two blocks ahead throughout.
            phase_a_open(0)
            for k in range(CT * NQ):
                phase_a_tile(0, k // CT, k % CT)
                # b1's loads issue during b0's pools: the Pool queue stays
                # short at the start and the DMA queue never goes idle
                emit_x_load(1, k % CT, k // CT)
            phase_a_finish(0)
            phase_a_open(1)
            phase_c_stage1(0, 0)
            phase_c_stage1(0, 1)
            for blk in range(NBLK // 2):
                for k in (2 * blk, 2 * blk + 1):
                    phase_a_tile(1, k // CT, k % CT)
                if blk + 2 < NBLK:
                    phase_c_stage1(0, blk + 2)
                phase_c_stage2(0, blk)
            for blk in range(NBLK // 2, NBLK):
                if blk + 2 < NBLK:
                    phase_c_stage1(0, blk + 2)
                phase_c_stage2(0, blk)
            phase_a_finish(1)
            phase_c(1)

    nc.finalize()
    return nc


def kernel(x, w1, w2):
    global _nc_cache
    if _nc_cache is None:
        _nc_cache = _build_nc()
    nc = _nc_cache

    from concourse.bass_utils import run_bass_kernel_spmd

    x = np.ascontiguousarray(np.asarray(x, dtype=np.float32))
    w1 = np.ascontiguousarray(np.asarray(w1, dtype=np.float32))
    w2 = np.ascontiguousarray(np.asarray(w2, dtype=np.float32))

    in_maps = [
        {"x": np.ascontiguousarray(x[i * BL:(i + 1) * BL]),
         "w1": w1, "w2": w2}
        for i in range(NCORES)
    ]
    res = run_bass_kernel_spmd(nc, in_maps, core_ids=list(range(NCORES)))
    return np.concatenate([r["out"] for r in res.results], axis=0)
